# revision 72
# baseline (speedup 1.0000x reference)
"""Multi-head cross attention (B=4, LQ=1024, LK=2048, D=1024, H=16) on 8 trn2 cores.

Sharding: batch (4-way) x head-group (2-way, 8 heads each). Each core computes a
partial output Y_part = softmax(Q_hg K_hg^T/sqrt(dh) + mask) V_hg @ W_O[hg rows];
host sums the two head-group partials per batch.

Key tricks:
  - Host compacts the key/value sequence to the unmasked keys (the reference
    mask kills ~half of them), padded to a multiple of 128; padded rows are
    zeros + a -1e30 exp-bias. Program is compiled per padded-chunk-count (and
    per output-bias-is-zero, which the reference's b_V/b_O are).
  - Host supplies q_input[b]^T / compacted kv^T in bf16 (feature-major), so
    projections contract over D with natural-layout weights; 1/sqrt(dh) is
    folded into W_Q/b_Q; b_K drops (softmax shift invariance); b_V@W_O + b_O
    becomes a host-computed output bias row. W_Q/W_K are host-swizzled
    chunk-major so per-chunk DMAs keep >=512B contiguous runs (smaller runs
    pay 2x on the wire).
  - Scores are computed transposed (S^T[k, q]) so the key mask is a
    per-partition bias folded into the exp on the scalar engine and P^T chunks
    feed the PV matmul directly (no on-chip transposes). Head pairs share a
    feature chunk at partitions 0-63 / 64-127.
  - V is augmented with a ones column per head; the PV matmul then emits the
    softmax denominators as PSUM row 64. Normalization uses a fast DVE
    reciprocal and a DRAM-bounce partition-broadcast mid-kernel (fully hidden),
    switching to an on-chip gpsimd partition_broadcast for the final stretch
    where the bounce's ~4us round trip would gate the last wo tiles.
  - Startup is wire-bound: the two HWDGE queues (sync + scalar) carry few,
    big, need-ordered DMAs (kv/q column-split so only the first 512 columns
    gate the first scores); the pool queue carries none so its engine is free.
    A junk-matmul warmup holds the PE p-state up while the first DMAs land.
  - Emission: qh0 of all head pairs first (projections ride as fillers inside
    the exp-paced j-loops, each pair carrying its own khat column tails), then
    qh1 of all pairs with the first half of the output projection spread
    across those stretches; the last stretch pre-accumulates wo chunks 0..2
    of q-rows 512:1024 into freed PSUM so only the c3 matmuls trail the final
    epilogue. PSUM->SBUF copies run on ACT (chunk 0 on DVE); gpsimd cannot
    touch PSUM on real HW.
"""

import math
import numpy as np
import ml_dtypes

import concourse.bass as bass
import concourse.mybir as mybir
from concourse import bacc
from concourse.tile import TileContext
from concourse.bass_utils import run_bass_kernel_spmd

BF16 = mybir.dt.bfloat16
F32 = mybir.dt.float32
NP_BF16 = ml_dtypes.bfloat16

B, LQ, LK, D = 4, 1024, 2048, 1024
H, DH = 16, 64
N_CORES = 8
HPC = 8            # heads per core
DC = HPC * DH      # 512 local feature dim
DCH = DC // 128    # 4 dc chunks (also head-pair count)
DK = D // 128      # 8 contraction chunks
QT = LQ // 128     # 8 query tiles
E = DH + 1         # augmented V width per head
NEG = -1.0e30

_CACHE = {}
COMPACT = True      # debug: host-side key compaction
FAST_RECIP = True   # debug: reciprocal_approx_fast vs exact reciprocal


def _build_program(KT, with_bias=True):
    """Build + compile the SPMD program for KT 128-wide key chunks.

    with_bias=False specializes for the all-zero output-bias case (the
    reference generates zero b_V/b_O): the final bias adds become plain
    PSUM->SBUF copies split across the ACT and DVE engines."""
    LKP = KT * 128
    nc = bacc.Bacc("TRN2", target_bir_lowering=False, debug=False, num_devices=N_CORES)

    qT_d = nc.dram_tensor("qT", [D, LQ], BF16, kind="ExternalInput")
    kvT_d = nc.dram_tensor("kvT", [D, LKP], BF16, kind="ExternalInput")
    # wq/wk host-swizzled chunk-major [c][p][d][n] so per-chunk DMAs have
    # 2KB contiguous runs on both sides (runs < 512B pay 2x on the wire)
    wq_d = nc.dram_tensor("wq", [DCH, 128, DK, 128], BF16, kind="ExternalInput")
    wk_d = nc.dram_tensor("wk", [DCH, 128, DK, 128], BF16, kind="ExternalInput")
    wv_d = nc.dram_tensor("wv", [D, DC], BF16, kind="ExternalInput")
    wo_d = nc.dram_tensor("wo", [DC, D], BF16, kind="ExternalInput")
    bq_d = nc.dram_tensor("bq", [DC], F32, kind="ExternalInput")
    mask_d = nc.dram_tensor("maskb", [LKP], F32, kind="ExternalInput")
    biasf_d = nc.dram_tensor("bias_f", [D], F32, kind="ExternalInput")
    y_d = nc.dram_tensor("y", [LQ, D], BF16, kind="ExternalOutput")
    rb_d = nc.dram_tensor("rbounce", [HPC, LQ], F32)  # recip bounce scratch

    qT_r = qT_d[:].rearrange("(c p) l -> p c l", p=128)
    kvT_r = kvT_d[:].rearrange("(c p) l -> p c l", p=128)
    wv_r = wv_d[:].rearrange("(c p) n -> p c n", p=128)

    with TileContext(nc) as tc:
        with (
            tc.tile_pool(name="consts", bufs=1) as consts,
            tc.tile_pool(name="ps", bufs=2, space="PSUM") as psum_big,
            tc.tile_pool(name="pso", bufs=4, space="PSUM") as psum_o,
            tc.tile_pool(name="exps", bufs=(10 if KT <= 12 else 8)) as exps_pool,
            tc.tile_pool(name="small", bufs=(8 if KT <= 10 else 4)) as small,
            tc.tile_pool(name="yout", bufs=(8 if KT <= 10 else 3)) as yout,
        ):
            # ---- constant loads: two HWDGE queues (sync + scalar), ordered by
            # first use. Sync carries the big streams (kvT, qT); scalar carries
            # the weights, split per output chunk so khat/qhat c0 unblock early.
            # gpsimd (Pool SWDGE, ~1us/descriptor) is kept free for the
            # PSUM->SBUF copies during the run.
            kvT_in = consts.tile([128, DK, LKP], BF16, name="kvT_in")
            wk_sb = consts.tile([128, DCH, DK, 128], BF16, name="wk_sb")
            wv_sb = consts.tile([128, DK, DC], BF16, name="wv_sb")
            qT_in = consts.tile([128, DK, LQ], BF16, name="qT_in")
            wq_sb = consts.tile([128, DCH, DK, 128], BF16, name="wq_sb")
            # The wire round-robins whole transfers across queues with no
            # priority, and HWDGE descriptor-gen is a shared ~630ns/DMA
            # resource: strict need-order, few big DMAs, nothing on pool
            # (SWDGE issue would occupy the pool engine needed for copies).
            mask_sb = consts.tile([128, KT], F32, name="mask_sb")
            bq_sb = consts.tile([128, DCH], F32, name="bq_sb")
            nc.scalar.dma_start(out=wk_sb[:, 0], in_=wk_d[0])
            nc.scalar.dma_start(out=wq_sb[:, 0], in_=wq_d[0])
            nc.scalar.dma_start(out=mask_sb, in_=mask_d[:].rearrange("(j p) -> p j", p=128))
            nc.scalar.dma_start(out=bq_sb, in_=bq_d[:].rearrange("(c p) -> p c", p=128))
            KA = min(512, LKP)
            for d in range(0, DK, 4):
                nc.sync.dma_start(out=kvT_in[:, d:d + 4, 0:KA], in_=kvT_r[:, d:d + 4, 0:KA])
            for d in range(0, DK, 4):
                nc.sync.dma_start(out=qT_in[:, d:d + 4, 0:512], in_=qT_r[:, d:d + 4, 0:512])
            nc.sync.dma_start(out=wv_sb[:, 0:4, :], in_=wv_r[:, 0:4, :])
            nc.sync.dma_start(out=wv_sb[:, 4:8, :], in_=wv_r[:, 4:8, :])
            if LKP > KA:
                for d in range(0, DK, 4):
                    nc.sync.dma_start(
                        out=kvT_in[:, d:d + 4, KA:LKP], in_=kvT_r[:, d:d + 4, KA:LKP]
                    )
            nc.sync.dma_start(out=qT_in[:, :, 512:1024], in_=qT_r[:, :, 512:1024])
            for c in range(1, DCH):
                nc.sync.dma_start(out=wk_sb[:, c], in_=wk_d[c])
                nc.sync.dma_start(out=wq_sb[:, c], in_=wq_d[c])
            wo_sb = consts.tile([128, DCH, D], BF16, name="wo_sb")
            nc.sync.dma_start(out=wo_sb, in_=wo_d[:].rearrange("(c p) n -> p c n", p=128))
            biasf_sb = consts.tile([128, D], F32, name="biasf_sb")
            bf_ap = biasf_d[:]
            nc.sync.dma_start(
                out=biasf_sb,
                in_=bass.AP(tensor=bf_ap.tensor, offset=bf_ap.offset, ap=[[0, 128]] + bf_ap.ap),
            )

            # ---- persistent intermediates ----
            qhatT = consts.tile([128, DCH, LQ], BF16, name="qhatT")     # [dc, lq]
            khatT = consts.tile([128, DCH, LKP], BF16, name="khatT")    # [dc, lk]
            v_sb = consts.tile([128, KT, HPC * E], BF16, name="v_sb")
            onormT = consts.tile([128, DCH, LQ], BF16, name="onormT")   # [dc, lq]

            # ones columns of augmented V (disjoint from the v copies below)
            nc.vector.memset(
                v_sb.rearrange("p t (h e) -> p t h e", e=E)[:, :, :, DH:DH + 1], 1.0
            )

            # PE p-state warmup: the tensor engine clock ramps only after ~3us
            # of continuous work. Junk matmuls on a zeroed tile keep it busy
            # (and ramping) while the first kv/q/weight DMAs stream in, so the
            # real projection matmuls start at full clock.
            warm = consts.tile([128, 512], BF16, name="warm")
            nc.vector.memset(warm[:], 0.0)
            _junk_n = [0]

            def junk(n):
                _junk_n[0] += 1
                wps = psum_o.tile([64, 512], F32, name=f"warm_ps{_junk_n[0]}", tag="pso")
                for i in range(n):
                    nc.tensor.matmul(
                        wps, lhsT=warm[:, 0:64], rhs=warm[:],
                        start=(i == 0), stop=(i == n - 1),
                    )

            junk(9)

            pre = {}  # (m, n0) -> pre-accumulated wo psum (chunks 0..2 done)

            def khat_slice(c, n0):
                w = min(512, LKP - n0)
                ps = psum_o.tile([128, w], F32, name=f"ps_k{c}_{n0}", tag="pso")
                for d in range(DK):
                    nc.tensor.matmul(
                        ps,
                        lhsT=wk_sb[:, c, d],
                        rhs=kvT_in[:, d, n0:n0 + w],
                        start=(d == 0), stop=(d == DK - 1),
                    )
                # gpsimd cannot read PSUM on HW; chunk 0 on DVE (free at
                # startup), the rest on ACT (prompt in the PE-dense stretches)
                if c == 0:
                    nc.vector.tensor_copy(out=khatT[:, c, n0:n0 + w], in_=ps)
                else:
                    nc.scalar.activation(
                        out=khatT[:, c, n0:n0 + w], in_=ps,
                        func=mybir.ActivationFunctionType.Copy,
                    )

            def khat_chunk(c):
                for n0 in range(0, LKP, 512):
                    khat_slice(c, n0)

            def qhat_half(c, nn):
                ps = psum_o.tile([128, 512], F32, name=f"ps_q{c}_{nn}", tag="pso")
                for d in range(DK):
                    nc.tensor.matmul(
                        ps,
                        lhsT=wq_sb[:, c, d],
                        rhs=qT_in[:, d, nn:nn + 512],
                        start=(d == 0), stop=(d == DK - 1),
                    )
                if c == 0:
                    nc.vector.tensor_scalar_add(
                        out=qhatT[:, c, nn:nn + 512], in0=ps, scalar1=bq_sb[:, c:c + 1]
                    )
                else:
                    nc.scalar.activation(
                        out=qhatT[:, c, nn:nn + 512], in_=ps,
                        func=mybir.ActivationFunctionType.Identity,
                        bias=bq_sb[:, c:c + 1], scale=1.0,
                    )

            def qhat_chunk(c):
                for nn in range(0, LQ, 512):
                    qhat_half(c, nn)

            def v_chunk_mm(t):
                ps = psum_o.tile([128, DC], F32, name=f"ps_v{t}", tag="pso")
                for d in range(DK):
                    nc.tensor.matmul(
                        ps,
                        lhsT=kvT_in[:, d, t * 128:(t + 1) * 128],
                        rhs=wv_sb[:, d, :],
                        start=(d == 0), stop=(d == DK - 1),
                    )
                return ps

            def v_chunk_copy(t, ps):
                nc.scalar.activation(
                    out=v_sb[:, t, :].rearrange("p (h e) -> p h e", e=E)[:, :, 0:DH],
                    in_=ps.rearrange("p (h e) -> p h e", e=DH),
                    func=mybir.ActivationFunctionType.Copy,
                )

            def attention_qh(hp, qh, emit_v=False, fillers=(), extra=(), bounce=True,
                             post_loop=None):
                """One query-half of a head pair. fillers: 0-arg callables
                emitting PE work popped between j iterations (keeps PE busy
                while ACT runs exp); extra: appended per-j after v_chunk."""
                fillers = list(fillers)
                extra = list(extra)
                h0, h1 = 2 * hp, 2 * hp + 1
                q0 = qh * 512
                opsA = psum_o.tile([E, 512], F32, name=f"opsA{hp}_{qh}", tag="pso")
                opsB = psum_o.tile([E, 512], F32, name=f"opsB{hp}_{qh}", tag="pso")
                for j in range(KT):
                    ps = psum_big.tile([128, 1024], F32, name=f"ps_s{hp}_{qh}_{j}", tag="ss")
                    # head pair in disjoint PE row groups
                    nc.tensor.matmul(
                        ps[:, 0:512],
                        lhsT=khatT[0:64, hp, j * 128:(j + 1) * 128],
                        rhs=qhatT[0:64, hp, q0:q0 + 512],
                        start=True, stop=True,
                    )
                    nc.tensor.matmul(
                        ps[:, 512:1024],
                        lhsT=khatT[64:128, hp, j * 128:(j + 1) * 128],
                        rhs=qhatT[64:128, hp, q0:q0 + 512],
                        start=True, stop=True,
                    )
                    v_ps = None
                    if emit_v:
                        v_ps = v_chunk_mm(j)  # fills PE while exp runs
                        if extra:
                            f = extra.pop(0)
                            if f is not None:
                                f()
                    elif fillers:
                        f = fillers.pop(0)
                        if f is not None:
                            f()
                    es = exps_pool.tile([128, 1024], BF16, name=f"es{hp}_{qh}_{j}", tag="es")
                    nc.scalar.activation(
                        out=es, in_=ps,
                        func=mybir.ActivationFunctionType.Exp,
                        bias=mask_sb[:, j:j + 1], scale=1.0,
                    )
                    if v_ps is not None:
                        v_chunk_copy(j, v_ps)  # after the exp: ACT HoL order
                    nc.tensor.matmul(
                        opsA,
                        lhsT=v_sb[:, j, h0 * E:(h0 + 1) * E],
                        rhs=es[:, 0:512],
                        start=(j == 0), stop=(j == KT - 1),
                    )
                    nc.tensor.matmul(
                        opsB,
                        lhsT=v_sb[:, j, h1 * E:(h1 + 1) * E],
                        rhs=es[:, 512:1024],
                        start=(j == 0), stop=(j == KT - 1),
                    )
                if post_loop is not None:
                    post_loop()
                if not bounce and post_loop is not None:
                    # final-stretch epilogue, engine-parallel (ACT is idle by
                    # now): ou copies on DVE+ACT,
                    # den staging on pool, recips+mults on DVE, on-chip
                    # broadcasts on pool (the DRAM bounce's ~4us round trip
                    # would gate the last wo tiles)
                    tst = []
                    for i, (h, po, ops) in enumerate(((h0, 0, opsA), (h1, 64, opsB))):
                        ou = small.tile([E, 512], F32, name=f"ou{h}_{qh}", tag="ou")
                        if i == 0:
                            nc.vector.tensor_copy(out=ou, in_=ops)
                        else:
                            nc.scalar.activation(
                                out=ou, in_=ops,
                                func=mybir.ActivationFunctionType.Copy,
                            )
                        den = small.tile([1, 512], F32, name=f"den{h}_{qh}", tag="den")
                        nc.gpsimd.tensor_copy(out=den, in_=ou[DH:DH + 1, :])
                        tst.append((po, ou, den))
                    recs = []
                    for po, ou, den in tst:
                        rec = small.tile([1, 512], F32, name=f"rec{po}_{qh}", tag="rec")
                        nc.vector.reciprocal_approx_fast(out=rec, in_=den)
                        recs.append(rec)
                    rbcs = []
                    for rec in recs:
                        rbc = small.tile([64, 512], F32, name=f"rbc{rec.name}", tag="rbc")
                        nc.gpsimd.partition_broadcast(rbc[:], rec[:], channels=64)
                        rbcs.append(rbc)
                    for (po, ou, den), rbc in zip(tst, rbcs):
                        nc.vector.tensor_mul(
                            out=onormT[po:po + 64, hp, q0:q0 + 512],
                            in0=ou[0:DH, :], in1=rbc,
                        )
                    for f in fillers + extra:
                        if f is not None:
                            f()
                    return
                # two-phase epilogue: all copies/recips/broadcasts first so the
                # per-head mults never head-of-line block the other head's chain
                stage = []
                for h, po, ops in ((h0, 0, opsA), (h1, 64, opsB)):
                    rec = small.tile([1, 512], F32, name=f"rec{h}_{qh}", tag="rec")
                    rbc = small.tile([64, 512], F32, name=f"rbc{h}_{qh}", tag="rbc")
                    if bounce:
                        # copy out of PSUM right away to free the slot early
                        ou = small.tile([E, 512], F32, name=f"ou{h}_{qh}", tag="ou")
                        nc.vector.tensor_copy(out=ou, in_=ops)
                        den = small.tile([1, 512], F32, name=f"den{h}_{qh}", tag="den")
                        nc.vector.tensor_copy(out=den, in_=ou[DH:DH + 1, :])
                        nc.vector.reciprocal_approx_fast(out=rec, in_=den)
                        nc.sync.dma_start(out=rb_d[h:h + 1, q0:q0 + 512], in_=rec)
                        rb_ap = rb_d[h:h + 1, q0:q0 + 512]
                        nc.sync.dma_start(
                            out=rbc,
                            in_=bass.AP(tensor=rb_ap.tensor, offset=rb_ap.offset,
                                        ap=[[0, 64], [1, 512]]),
                        )
                        src = ou[0:DH, :]
                    else:
                        # tail: on-chip broadcast (pool is idle; the DRAM
                        # bounce's ~4us round trip would gate the last wo tiles)
                        ou = small.tile([E, 512], F32, name=f"ou{h}_{qh}", tag="ou")
                        nc.vector.tensor_copy(out=ou, in_=ops)
                        den = small.tile([1, 512], F32, name=f"den{h}_{qh}", tag="den")
                        nc.vector.tensor_copy(out=den, in_=ou[DH:DH + 1, :])
                        nc.vector.reciprocal_approx_fast(out=rec, in_=den)
                        nc.gpsimd.partition_broadcast(rbc[:], rec[:], channels=64)
                        src = ou[0:DH, :]
                    stage.append((po, src, rbc))
                for po, src, rbc in stage:
                    nc.vector.tensor_mul(
                        out=onormT[po:po + 64, hp, q0:q0 + 512],
                        in0=src, in1=rbc,
                    )
                for f in fillers + extra:
                    if f is not None:
                        f()

            def wo_half(m, n):
                ys = yout.tile([128, 512], BF16, name=f"ys{m}_{n}", tag="ys")
                if (m, n * 512) in pre:
                    ps = pre.pop((m, n * 512))
                    nc.tensor.matmul(
                        ps,
                        lhsT=onormT[:, DCH - 1, m * 128:(m + 1) * 128],
                        rhs=wo_sb[:, DCH - 1, n * 512:(n + 1) * 512],
                        start=False, stop=True,
                    )
                else:
                    ps = psum_o.tile([128, 512], F32, name=f"ps_y{m}_{n}", tag="pso")
                    for c in range(DCH):
                        nc.tensor.matmul(
                            ps,
                            lhsT=onormT[:, c, m * 128:(m + 1) * 128],
                            rhs=wo_sb[:, c, n * 512:(n + 1) * 512],
                            start=(c == 0), stop=(c == DCH - 1),
                        )
                if with_bias:
                    nc.vector.tensor_add(
                        out=ys, in0=ps, in1=biasf_sb[:, n * 512:(n + 1) * 512],
                    )
                else:
                    nc.vector.tensor_copy(out=ys, in_=ps)
                # spread the output stream over the free queues so the per-DMA
                # ~2.5us queue hold doesn't serialize the tail. scalar only for
                # m>=4 (its exps are done by then)
                if m < 4:
                    eng = (nc.sync, nc.gpsimd)[(m + n) % 2]
                else:
                    eng = (nc.sync, nc.scalar, nc.gpsimd)[(2 * m + n) % 3]
                eng.dma_start(
                    out=y_d[m * 128:(m + 1) * 128, n * 512:(n + 1) * 512], in_=ys
                )

            def wo_pre(ms, pool, width, tag):
                def f():
                    for m in ms:
                        for n0 in range(0, 1024, width):
                            ps = pool.tile([128, width], F32,
                                           name=f"ps_w{m}_{n0}", tag=tag)
                            for c in range(DCH - 1):
                                # one matmul per 512 cols: a single matmul's
                                # output must stay within one PSUM bank
                                for s0 in range(0, width, 512):
                                    nc.tensor.matmul(
                                        ps[:, s0:s0 + 512],
                                        lhsT=onormT[:, c, m * 128:(m + 1) * 128],
                                        rhs=wo_sb[:, c, n0 + s0:n0 + s0 + 512],
                                        start=(c == 0), stop=False,
                                    )
                            pre[(m, n0)] = ps
                return f

            # ---- emission order: chunk-0 khat/qhat first (only slice n0=0 /
            # half nn=0 are needed for scores j=0); the rest of chunk 0 plus the
            # next chunk's projections ride as fillers inside the j-loops ----
            def proj_fillers(c):
                f = [lambda n0=n0: khat_slice(c, n0) for n0 in range(0, LKP, 512)]
                f += [lambda nn=nn: qhat_half(c, nn) for nn in range(0, LQ, 512)]
                return f

            khat_slice(0, 0)
            junk(5)
            qhat_half(0, 0)
            junk(4)
            wof_ps = {}

            def wo_unit(m, n, c):
                def f():
                    if c == 0:
                        wof_ps[(m, n)] = psum_o.tile(
                            [128, 512], F32, name=f"ps_y{m}_{n}", tag="pso")
                    ps = wof_ps[(m, n)]
                    nc.tensor.matmul(
                        ps,
                        lhsT=onormT[:, c, m * 128:(m + 1) * 128],
                        rhs=wo_sb[:, c, n * 512:(n + 1) * 512],
                        start=(c == 0), stop=(c == DCH - 1),
                    )
                    if c == DCH - 1:
                        ps = wof_ps.pop((m, n))
                        ys = yout.tile([128, 512], BF16, name=f"ys{m}_{n}", tag="ys")
                        if with_bias:
                            nc.vector.tensor_add(
                                out=ys, in0=ps, in1=biasf_sb[:, n * 512:(n + 1) * 512])
                        else:
                            nc.vector.tensor_copy(out=ys, in_=ps)
                        eng = (nc.sync, nc.gpsimd)[(m + n) % 2]
                        eng.dma_start(
                            out=y_d[m * 128:(m + 1) * 128, n * 512:(n + 1) * 512],
                            in_=ys)
                return f

            # qh0 of every pair first, then qh1 of every pair. Each pair's qh0
            # carries its OWN khat column tails (slice n0=512 is first read at
            # j=4, n0=1024 at j=8) plus the next chunk's head slices, so every
            # stretch has PE filler work. pair-0's list is ordered by DMA
            # arrival (kv/q column tails land before the chunk-1 weights).
            tail0 = [lambda n0=n0: khat_slice(0, n0) for n0 in range(512, LKP, 512)]
            tail0 += [lambda: qhat_half(0, 512)]
            tail0 += [lambda: khat_slice(1, 0), lambda: qhat_half(1, 0)]
            attention_qh(0, 0, emit_v=True, extra=tail0)
            for c in range(1, DCH):
                f = [lambda n0=n0: khat_slice(c, n0) for n0 in range(512, LKP, 512)]
                if c + 1 < DCH:
                    f += [lambda: khat_slice(c + 1, 0), lambda: qhat_half(c + 1, 0)]
                f += [lambda: qhat_half(c, 512)]
                if c == DCH - 1:
                    # idle stretch: chunks 0..2 of onormT's qh0 half are final
                    f += [wo_unit(0, n, cc) for n in range(2) for cc in range(DCH - 1)]
                attention_qh(c, 0, fillers=f, bounce=(c < DCH - 1))
            wof = [wo_unit(0, 0, DCH - 1), wo_unit(0, 1, DCH - 1)]
            wof += [wo_unit(m, n, cc) for m in range(1, 4) for n in range(2)
                    for cc in range(DCH)]
            attention_qh(0, 1, fillers=[None, None] + wof[0:7])
            attention_qh(1, 1, fillers=wof[7:16])
            attention_qh(2, 1, fillers=wof[16:26])

            # last stretch: pre-accumulate wo rows 512:768 over chunks 0..2 in
            # the psum_big slots freed by the final exps, and rows 768:1024
            # in the psum_o slots freed by the ou copies (they only need the
            # qh0-half onormT). After this stretch's epilogue only the c3
            # matmul + a rank-1 bias term + PSUM->SBUF copies remain, with the
            # copies alternating ACT/DVE so neither engine serializes the tail.

            def wo_fin(m):
                # one [128, 1024] ys tile + one DMA per m-tile: fewer init
                # latencies in the drain; halves still copy on ACT/DVE in
                # parallel streams
                ysm = yout.tile([128, 1024], BF16, name=f"ysm{m}", tag="ys")
                for n0 in sorted(n0 for (mm, n0) in pre if mm == m):
                    ps = pre.pop((m, n0))
                    w = ps.shape[-1]
                    for s0 in range(0, w, 512):
                        nc.tensor.matmul(
                            ps[:, s0:s0 + 512],
                            lhsT=onormT[:, DCH - 1, m * 128:(m + 1) * 128],
                            rhs=wo_sb[:, DCH - 1, n0 + s0:n0 + s0 + 512],
                            start=False, stop=True,
                        )
                    for nn in range(n0, n0 + w, 512):
                        k = 2 * m + nn // 512
                        if with_bias:
                            nc.vector.tensor_add(
                                out=ysm[:, nn:nn + 512], in0=ps[:, nn - n0:nn - n0 + 512],
                                in1=biasf_sb[:, nn:nn + 512],
                            )
                        elif k % 2 == 0:
                            nc.scalar.activation(
                                out=ysm[:, nn:nn + 512], in_=ps[:, nn - n0:nn - n0 + 512],
                                func=mybir.ActivationFunctionType.Copy,
                            )
                        else:
                            nc.vector.tensor_copy(
                                out=ysm[:, nn:nn + 512], in_=ps[:, nn - n0:nn - n0 + 512]
                            )
                # late tiles (m5, m7) ride the fast HWDGE sync queue; the
                # early ones take the pool SWDGE whose ~1us engine cost hides
                eng = (nc.gpsimd, nc.sync)[m % 2]
                eng.dma_start(out=y_d[m * 128:(m + 1) * 128, :], in_=ysm)

            def post_loop():
                wo_pre((4, 5), psum_big, 1024, "ss")()

            attention_qh(3, 1, bounce=False, post_loop=post_loop)
            wo_pre((6, 7), psum_o, 512, "pso")()
            for m in (4, 5, 6, 7):
                wo_fin(m)

    nc.compile()
    return nc


def _get_program(KT, with_bias=True):
    key = ("nc", KT, with_bias)
    if key not in _CACHE:
        _CACHE[key] = _build_program(KT, with_bias)
    return _CACHE[key]


def kernel(q_input, kv_input, key_padding_mask, W_Q, b_Q, W_K, b_K, W_V, b_V, W_O, b_O):
    q_input = np.asarray(q_input, dtype=np.float32)
    kv_input = np.asarray(kv_input, dtype=np.float32)
    key_padding_mask = np.asarray(key_padding_mask).astype(bool)
    W_Q = np.asarray(W_Q, dtype=np.float32)
    b_Q = np.asarray(b_Q, dtype=np.float32)
    W_K = np.asarray(W_K, dtype=np.float32)
    W_V = np.asarray(W_V, dtype=np.float32)
    b_V = np.asarray(b_V, dtype=np.float32)
    W_O = np.asarray(W_O, dtype=np.float32)
    b_O = np.asarray(b_O, dtype=np.float32)

    q_bf = q_input.astype(NP_BF16)
    kv_bf = kv_input.astype(NP_BF16)

    # compact keys/values to the unmasked rows, pad to a 128 multiple
    if COMPACT:
        keeps = [~key_padding_mask[b] for b in range(B)]
    else:
        keeps = [np.ones(LK, bool) for _ in range(B)]
    effs = [int(k.sum()) for k in keeps]
    KT = max(1, math.ceil(max(effs) / 128))
    LKP = KT * 128
    with_bias = bool(np.any(b_V) or np.any(b_O))
    nc = _get_program(KT, with_bias)

    # per head-group constants
    def _swz(w):
        # [D, DC] -> chunk-major [c, p, d, n] blocks (2KB contiguous runs)
        return np.ascontiguousarray(
            w.reshape(DK, 128, DCH, 128).transpose(2, 1, 0, 3)
        )

    hg_const = []
    for hg in range(2):
        sl = slice(hg * DC, (hg + 1) * DC)
        wq = _swz((W_Q[:, sl] * 0.125).astype(NP_BF16))
        wk = _swz(W_K[:, sl].astype(NP_BF16))
        wv = np.ascontiguousarray(W_V[:, sl].astype(NP_BF16))
        wo = np.ascontiguousarray(W_O[sl, :].astype(NP_BF16))
        bq = (b_Q[sl] * 0.125).astype(np.float32)
        bias_f = (b_V[sl].astype(np.float64) @ W_O[sl, :].astype(np.float64)).astype(np.float32)
        if hg == 0:
            bias_f = bias_f + b_O
        hg_const.append((wq, wk, wv, wo, bq, bias_f))

    per_batch = []
    for b in range(B):
        kvc = kv_bf[b][keeps[b]]            # [eff, D]
        kvT = np.zeros((D, LKP), NP_BF16)
        kvT[:, :effs[b]] = kvc.T
        maskb = np.full(LKP, np.float32(NEG), np.float32)
        maskb[:effs[b]] = np.where(key_padding_mask[b][keeps[b]], np.float32(NEG), np.float32(0.0))
        per_batch.append((np.ascontiguousarray(q_bf[b].T), kvT, maskb))

    in_maps = []
    for core in range(N_CORES):
        b, hg = core // 2, core % 2
        wq, wk, wv, wo, bq, bias_f = hg_const[hg]
        qT, kvT, maskb = per_batch[b]
        in_maps.append({
            "qT": qT, "kvT": kvT,
            "wq": wq, "wk": wk, "wv": wv, "wo": wo,
            "bq": bq, "maskb": maskb, "bias_f": bias_f,
        })

    _CACHE["in_maps"] = in_maps
    _CACHE["last_KT"] = KT
    _CACHE["last_with_bias"] = with_bias
    res = run_bass_kernel_spmd(nc, in_maps, core_ids=list(range(N_CORES)))
    out = np.stack(
        [res.results[2 * b]["y"].astype(np.float32) + res.results[2 * b + 1]["y"].astype(np.float32) for b in range(B)]
    )
    return out.astype(np.float32)

